# revision 2
# baseline (speedup 1.0000x reference)
"""Trainium2 Bass kernel for the Dial2vec contrastive loss (nn_Dial2vec).

Math (see v2 notes): per sequence, with turn one-hots folded through the
band matrix,
    Gt[hb]  = h^T @ ABX            [128h, 32]  (= [Band R_T ; Band Q_T]^T)
    U       = hxt^T @ Gt           [128tok, 32]
    gam     = rowsum(U ∘ AB2)      per token
    out     = [a, b, a*gam, b*gam]^T @ h  ->  qs/rs/qc/rc rows
followed by a host-side O(B*H) cosine / log-softmax reduction.  Cosine
similarity is scale-invariant, so mask-count denominators and the global
2^-6 scale on gam cancel.

v3 layout: sequences are packed to SLC=288 tokens (the data's max active
count is 284, so 384-padding wasted 25% of the h payload, which is shipped
twice).  Seq boundaries sit at fixed offsets 288*s, so chunk/seq incidence
structure is compile-time constant and identical on all 8 cores (one SPMD
program).  Boundary chunks are handled data-side: per-incidence abx/ab2
with foreign-token rows zeroed, so every matmul runs full 128 partitions.

The D-pass uses one wide [40, 768] psum accumulation over all 23 chunks
(lhsT = 40 per-seq mask columns: 20 static a/b pairs + 20 gam-scaled pairs
written on device), replacing per-seq 32-row strips: 23x768 col-streams
instead of 30x768, and a 61KB output.

DMA: one big per-core DRAM row [128, MROW], loaded via ~17 ordered
dma_starts all issued from the otherwise-idle sync sequencer, interleaving
hx/hxt chunk-groups so the G-pass starts ~1.6us in and the U-pass trails
by two chunks.
"""

import os

import numpy as np

B_SEQ = 80
L = 512
H = 768
SAMPLES = 10
T = 16
VIEW_RANGE = 2
TEMP = 0.2
AVG_EPS = 1e-6
COS_EPS = 1e-8

N_CORES = 8
SPC = SAMPLES  # sequences per core = one dialogue
P = 128
HB = H // P  # 6 h-blocks
TT = 2 * T  # 32
GSC = 2.0**-6  # keeps a*gam within fp8e4 range; cancels in the cosine
SLC_DEFAULT = 288  # per-seq packed token budget (max active count is 284)

_CACHE: dict = {}


def _structure(slc):
    """Fixed chunk/sequence incidence structure for a given per-seq budget."""
    nt = slc * SPC
    nch = (nt + P - 1) // P
    ntp = nch * P
    chunks_of_seq = [
        list(range((slc * s) // P, (slc * (s + 1) - 1) // P + 1)) for s in range(SPC)
    ]
    incs = []  # chunk-major (c, s) pairs
    inc_of = {}
    for c in range(nch):
        for s in range(SPC):
            if c in chunks_of_seq[s]:
                inc_of[(c, s)] = len(incs)
                incs.append((c, s))
    s_last_of_chunk = {c: max(s for (cc, s) in incs if cc == c) for c in range(nch)}
    ni = len(incs)
    off_abx = 0
    off_ab2 = ni * TT
    off_abd = 2 * ni * TT
    off_hx = off_abd + nch * 40
    off_hxt = off_hx + nch * H
    mrow = off_hxt + nch * H
    return dict(
        slc=slc, nt=nt, nch=nch, ntp=ntp, chunks_of_seq=chunks_of_seq,
        incs=incs, inc_of=inc_of, s_last_of_chunk=s_last_of_chunk, ni=ni,
        off_abx=off_abx, off_ab2=off_ab2, off_abd=off_abd, off_hx=off_hx,
        off_hxt=off_hxt, mrow=mrow,
    )


def _build_nc(st):
    from contextlib import ExitStack

    import concourse.bacc as bacc
    import concourse.mybir as mybir
    import concourse.tile as tile

    f32 = mybir.dt.float32
    bf16 = mybir.dt.bfloat16
    f8 = mybir.dt.float8e4

    nch, ni = st["nch"], st["ni"]
    incs, inc_of = st["incs"], st["inc_of"]
    chunks_of_seq = st["chunks_of_seq"]
    s_last_of_chunk = st["s_last_of_chunk"]
    OFF_ABX, OFF_AB2, OFF_ABD = st["off_abx"], st["off_ab2"], st["off_abd"]
    OFF_HX, OFF_HXT, MROW = st["off_hx"], st["off_hxt"], st["mrow"]

    nc = bacc.Bacc(
        "TRN2",
        debug=False,
        enable_asserts=False,
        target_bir_lowering=False,
    )

    mg = nc.dram_tensor("mg", [P, MROW], f8, kind="ExternalInput").ap()
    out = nc.dram_tensor("out", [4 * SPC, H], bf16, kind="ExternalOutput").ap()

    with tile.TileContext(nc) as tc, ExitStack() as ctx:
        mgp = ctx.enter_context(tc.tile_pool(name="mgp", bufs=1))
        gtp = ctx.enter_context(tc.tile_pool(name="gtp", bufs=5))
        scp = ctx.enter_context(tc.tile_pool(name="scp", bufs=4))
        gmp = ctx.enter_context(tc.tile_pool(name="gmp", bufs=1))
        osp = ctx.enter_context(tc.tile_pool(name="osp", bufs=1))
        pgp = ctx.enter_context(tc.tile_pool(name="pgp", bufs=3, space="PSUM"))
        pup = ctx.enter_context(tc.tile_pool(name="pup", bufs=4, space="PSUM"))
        pdp = ctx.enter_context(tc.tile_pool(name="pdp", bufs=1, space="PSUM"))

        mgt = mgp.tile([P, MROW], f8, name="mg", tag="mg")
        gam = gmp.tile([P, ni], f32, name="gam", tag="ga")

        # ---- DMA schedule: ordered starts, all on the sync sequencer ----
        # (sync has no other duties; descriptor spray across the 16 rings
        # drains starts roughly in issue order, so order = priority)
        def span(a, b):
            nc.sync.dma_start(mgt[:, a:b], mg[:, a:b])

        hx_groups = [(0, 2), (2, 4), (4, 8), (8, 12), (12, 16), (16, 20), (20, nch)]
        hxt_groups = hx_groups
        span(OFF_ABX, OFF_ABD)  # abx + ab2 masks, needed by G(0)/STT
        span(OFF_HX, OFF_HX + 2 * H)  # hx chunks 0-1
        span(OFF_ABD, OFF_HX)  # abd static cols
        for i in range(1, len(hx_groups)):
            a, b = hx_groups[i]
            nc.sync.dma_start(
                mgt[:, OFF_HX + a * H : OFF_HX + b * H],
                mg[:, OFF_HX + a * H : OFF_HX + b * H],
            )
            a, b = hxt_groups[i - 1]
            nc.sync.dma_start(
                mgt[:, OFF_HXT + a * H : OFF_HXT + b * H],
                mg[:, OFF_HXT + a * H : OFF_HXT + b * H],
            )
        a, b = hxt_groups[-1]
        span(OFF_HXT + a * H, OFF_HXT + b * H)

        # ---- compute passes ----
        def g_pass(s):
            pg = pgp.tile([P, HB * TT], f32, name=f"pg{s}", tag="pg")
            cs = chunks_of_seq[s]
            for hb in range(HB):
                for k, c in enumerate(cs):
                    nc.tensor.matmul(
                        pg[:, hb * TT : (hb + 1) * TT],
                        mgt[:, OFF_HX + c * H + hb * P : OFF_HX + c * H + (hb + 1) * P],
                        mgt[:, OFF_ABX + inc_of[(c, s)] * TT : OFF_ABX + (inc_of[(c, s)] + 1) * TT],
                        start=(k == 0),
                        stop=(k == len(cs) - 1),
                        skip_group_check=True,
                    )
            gt = gtp.tile([P, HB * TT], f8, name=f"gt{s}", tag="gt")
            half = HB * TT // 2
            nc.vector.tensor_copy(gt[:, 0:half], pg[:, 0:half])
            nc.scalar.copy(gt[:, half:], pg[:, half:])
            return gt

        def u_pass(c, gts):
            """U + gam + abd scale for all incidences of chunk c."""
            for j, s in enumerate(s_list(c)):
                i = inc_of[(c, s)]
                pu = pup.tile([P, TT], f32, name=f"pu{i}", tag="pu")
                for hb in range(HB):
                    nc.tensor.matmul(
                        pu,
                        mgt[:, OFF_HXT + c * H + hb * P : OFF_HXT + c * H + (hb + 1) * P],
                        gts[s][:, hb * TT : (hb + 1) * TT],
                        start=(hb == 0),
                        stop=(hb == HB - 1),
                        skip_group_check=True,
                    )
                scr = scp.tile([P, TT], bf16, name=f"sc{i}", tag="sc")
                nc.vector.scalar_tensor_tensor(
                    out=scr,
                    in0=pu,
                    scalar=1.0,
                    in1=mgt[:, OFF_AB2 + i * TT : OFF_AB2 + (i + 1) * TT],
                    op0=mybir.AluOpType.bypass,
                    op1=mybir.AluOpType.mult,
                    accum_out=gam[:, i : i + 1],
                )
                # abd dynamic cols: first incidence covers the whole 20-col
                # block (foreign seqs' rows come out zero since either their
                # static cols or this gam column are zero); later incidences
                # patch only their own 2 columns.
                base = OFF_ABD + c * 40
                if j == 0:
                    nc.gpsimd.tensor_scalar_mul(
                        mgt[:, base + 20 : base + 40],
                        mgt[:, base : base + 20],
                        gam[:, i : i + 1],
                    )
                else:
                    nc.gpsimd.tensor_scalar_mul(
                        mgt[:, base + 20 + 2 * s : base + 22 + 2 * s],
                        mgt[:, base + 2 * s : base + 2 + 2 * s],
                        gam[:, i : i + 1],
                    )

        def s_list(c):
            return sorted(s for (cc, s) in incs if cc == c)

        pd0 = pdp.tile([4 * SPC, 512], f32, name="pd0", tag="pd")
        pd1 = pdp.tile([4 * SPC, H - 512], f32, name="pd1", tag="pd")

        def d_pass(c):
            for pd, n0, n1 in ((pd0, 0, 512), (pd1, 512, H)):
                nc.tensor.matmul(
                    pd,
                    mgt[:, OFF_ABD + c * 40 : OFF_ABD + (c + 1) * 40],
                    mgt[:, OFF_HX + c * H + n0 : OFF_HX + c * H + n1],
                    start=(c == 0),
                    stop=(c == nch - 1),
                    skip_group_check=True,
                )

        # ---- software pipeline ----
        # G is seq-major; U(c) fires once the last seq overlapping c has its
        # gt ready; D trails U by 2 chunks to hide the DVE/gpsimd gam chain.
        gts = {}
        u_done = 0
        d_done = 0

        def drain_u(limit):
            nonlocal u_done
            while u_done < limit:
                u_pass(u_done, gts)
                u_done += 1

        def drain_d(limit):
            nonlocal d_done
            while d_done < limit:
                d_pass(d_done)
                d_done += 1

        for s in range(SPC):
            gts[s] = g_pass(s)
            ready = max(c for c in range(nch) if s_last_of_chunk[c] <= s) + 1
            drain_u(ready)
            drain_d(max(0, u_done - 2))
        drain_u(nch)
        drain_d(nch)

        ot = osp.tile([4 * SPC, H], bf16, name="ot", tag="ot")
        nc.scalar.copy(ot[:, 0:512], pd0)
        nc.vector.tensor_copy(ot[:, 512:H], pd1)
        nc.sync.dma_start(out, ot)

    nc.compile()
    return nc


def _prep_core_inputs(hidden_states, attention_mask, role_ids, turn_ids, st):
    """Per-core packed input rows: index-only mask prep + fp8 quantize."""
    import ml_dtypes

    f8 = ml_dtypes.float8_e4m3

    slc, nch, ntp, ni = st["slc"], st["nch"], st["ntp"], st["ni"]
    incs = st["incs"]

    active = attention_mask != 0
    counts = active.sum(-1)
    assert counts.max() <= slc, f"active tokens {counts.max()} exceed SLC={slc}"
    # stable-sort active tokens to the front; padded positions carry zero masks
    sel = np.argsort(~active, axis=1, kind="stable")[:, :slc]

    am = np.take_along_axis(active, sel, axis=1).astype(np.float32)
    ro = np.take_along_axis(role_ids, sel, axis=1)
    tu = np.take_along_axis(turn_ids, sel, axis=1)
    hc = np.take_along_axis(hidden_states, sel[..., None], axis=1)

    a = am * (ro == 0)
    b = am * (ro == 1)
    onehot = (tu[..., None] == np.arange(T, dtype=tu.dtype)).astype(np.float32)
    A1 = onehot * a[..., None]
    B1 = onehot * b[..., None]
    band = (
        np.abs(np.arange(T)[:, None] - np.arange(T)[None, :]) <= VIEW_RANGE
    ).astype(np.float32)
    ABX = np.concatenate([B1 @ band, A1 @ band], axis=-1)  # [B, slc, 32]
    AB2 = np.concatenate([A1, B1], axis=-1)

    hq = hc.astype(f8)  # quantize once; all views share the same values
    hq32 = hq.astype(np.float32)

    in_maps = []
    for core in range(N_CORES):
        sl = slice(core * SPC, (core + 1) * SPC)
        # packed token stream [ntp, ...] with seq s at rows slc*s
        hs = np.zeros((ntp, H), np.float32)
        abxs = np.zeros((ntp, TT), np.float32)
        ab2s = np.zeros((ntp, TT), np.float32)
        as_ = np.zeros(ntp, np.float32)
        bs_ = np.zeros(ntp, np.float32)
        for s in range(SPC):
            g = core * SPC + s
            hs[slc * s : slc * (s + 1)] = hq32[g]
            abxs[slc * s : slc * (s + 1)] = ABX[g]
            ab2s[slc * s : slc * (s + 1)] = AB2[g]
            as_[slc * s : slc * (s + 1)] = a[g]
            bs_[slc * s : slc * (s + 1)] = b[g]

        # per-incidence abx/ab2: chunk c rows, foreign-seq rows zeroed
        abx_i = np.zeros((P, ni * TT), np.float32)
        ab2_i = np.zeros((P, ni * TT), np.float32)
        for i, (c, s) in enumerate(incs):
            rows = np.arange(c * P, (c + 1) * P)
            inseq = (rows >= slc * s) & (rows < slc * (s + 1))
            abx_i[:, i * TT : (i + 1) * TT] = abxs[rows] * inseq[:, None]
            ab2_i[:, i * TT : (i + 1) * TT] = ab2s[rows] * inseq[:, None]

        # abd: [128, nch*40]; per chunk: 20 static cols (a,b)*GSC at 2s,
        # 20 dynamic cols (device-written)
        abd = np.zeros((P, nch * 40), np.float32)
        for c in range(nch):
            rows = np.arange(c * P, (c + 1) * P)
            svec = rows // slc  # owning seq per row (pad rows clamp to 9)
            svec = np.minimum(svec, SPC - 1)
            cols0 = c * 40 + 2 * svec
            abd[np.arange(P), cols0] = as_[rows] * GSC
            abd[np.arange(P), cols0 + 1] = bs_[rows] * GSC

        # hx: [128, nch*768]; hx[p, c*H + j] = h[c*128+p, j]
        hx = hs.reshape(nch, P, H).transpose(1, 0, 2).reshape(P, nch * H)
        # hxt: [128, nch*768]; hxt[p, c*H + hb*P + t] = h[c*128+t, hb*P+p]
        hxt = (
            hs.reshape(nch, P, HB, P)
            .transpose(3, 0, 2, 1)  # [p, c, hb, t]
            .reshape(P, nch * H)
        )

        mgall = np.concatenate(
            [
                abx_i.astype(f8).view(np.uint8),
                ab2_i.astype(f8).view(np.uint8),
                abd.astype(f8).view(np.uint8),
                hx.astype(f8).view(np.uint8),
                hxt.astype(f8).view(np.uint8),
            ],
            axis=-1,
        ).view(f8)
        assert mgall.shape == (P, st["mrow"])
        in_maps.append({"mg": np.ascontiguousarray(mgall)})

    # cheap integrity reference: qs/rs rows recomputed on host from the same fp8 h
    qs_ref = np.einsum("bl,blh->bh", a, hq32) * GSC
    rs_ref = np.einsum("bl,blh->bh", b, hq32) * GSC
    return in_maps, a.sum(-1), b.sum(-1), qs_ref, rs_ref


def _outputs_ok(vecs, qs_ref, rs_ref):
    """Detect corrupted device runs: finite outputs + qs/rs rows match host."""
    if not np.isfinite(vecs).all():
        return False
    for got, ref in ((vecs[:, 0], qs_ref), (vecs[:, 1], rs_ref)):
        num = np.linalg.norm(got - ref, axis=-1)
        den = np.linalg.norm(ref, axis=-1) + 1e-6
        if (num / den).max() > 0.05:
            return False
    return True


def _finalize(vecs, labels, na, nb):
    """Host-side O(B*H) reduction: cosine, log-softmax, label-weighted loss.

    vecs: [B, 4, H] rows [qs, rs, qc, rc] (uniform scales cancel in cosine).
    """
    vecs = vecs.astype(np.float64)
    qs = vecs[:, 0] / (na + AVG_EPS)[:, None]
    rs = vecs[:, 1] / (nb + AVG_EPS)[:, None]
    qc = vecs[:, 2] / (nb + AVG_EPS)[:, None]
    rc = vecs[:, 3] / (na + AVG_EPS)[:, None]

    def cos(x, y):
        nx = np.maximum(np.linalg.norm(x, axis=-1), COS_EPS)
        ny = np.maximum(np.linalg.norm(y, axis=-1), COS_EPS)
        return (x * y).sum(-1) / (nx * ny)

    logit_q = (cos(qs, qc) / TEMP).reshape(-1, SAMPLES)
    logit_r = (cos(rs, rc) / TEMP).reshape(-1, SAMPLES)

    def lsm(x):
        m = x.max(-1, keepdims=True)
        e = np.exp(x - m)
        return x - m - np.log(e.sum(-1, keepdims=True))

    lab = labels.astype(np.float64)
    loss_q = -np.mean(lsm(logit_q) * lab)
    loss_r = -np.mean(lsm(logit_r) * lab)
    return np.float32(loss_r + loss_q)


def kernel(hidden_states, labels, attention_mask, role_ids, turn_ids):
    import time

    from concourse.bass_utils import run_bass_kernel_spmd

    attention_mask = np.asarray(attention_mask)
    maxc = int((attention_mask != 0).sum(-1).max())
    slc = max(SLC_DEFAULT, -(-maxc // 32) * 32)
    if ("nc", slc) not in _CACHE:
        _CACHE[("nc", slc)] = _build_nc(_structure(slc))
    nc = _CACHE[("nc", slc)]
    st = _structure(slc)

    in_maps, na, nb, qs_ref, rs_ref = _prep_core_inputs(
        np.asarray(hidden_states),
        attention_mask,
        np.asarray(role_ids),
        np.asarray(turn_ids),
        st,
    )
    trace = bool(os.environ.get("BASS_KERNEL_TRACE"))

    # the axon/NRT path very occasionally drops a run; validate cheaply and retry
    vecs = None
    for attempt in range(3):
        try:
            res = run_bass_kernel_spmd(
                nc, in_maps, core_ids=list(range(N_CORES)), trace=trace
            )
            cand = np.concatenate(
                [res.results[c]["out"].reshape(SPC * 2, 2, H) for c in range(N_CORES)],
                axis=0,
            )  # per core rows: [2s]=qs, [2s+1]=rs then [20+2s]=qc, [21+2s]=rc
            cand = (
                cand.reshape(N_CORES, 2, SPC, 2, H)
                .transpose(0, 2, 3, 1, 4)
                .reshape(B_SEQ, 4, H)
            )  # -> [b, (qs, qc, rs, rc)? see below]
            # rows come out as [qs, qc, rs, rc]; reorder to [qs, rs, qc, rc]
            cand = cand[:, [0, 2, 1, 3], :]
        except Exception as e:
            import traceback

            print(f"[kernel] attempt {attempt} failed: {type(e).__name__}: {e}")
            traceback.print_exc()
            if attempt == 2:
                raise
            time.sleep(2.0)
            continue
        vecs = cand
        if _outputs_ok(cand, qs_ref, rs_ref):
            break
    if trace:
        _CACHE["last_results"] = res
        print(
            f"[kernel] exec_time_ns={res.exec_time_ns} "
            f"mean_exec_time_ns={res.mean_exec_time_ns}"
        )
    return _finalize(vecs, np.asarray(labels), na, nb)


# revision 6
# speedup vs baseline: 1.2020x; 1.2020x over previous
"""Trainium2 Bass kernel for the Dial2vec contrastive loss (nn_Dial2vec).

Math (see v2 notes): per sequence, with turn one-hots folded through the
band matrix,
    Gt[hb]  = h^T @ ABX            [128h, 32]  (= [Band R_T ; Band Q_T]^T)
    U       = hxt^T @ Gt           [128tok, 32]
    gam     = rowsum(U ∘ AB2)      per token
    out     = [a, b, a*gam, b*gam]^T @ h  ->  qs/rs/qc/rc rows
followed by a host-side O(B*H) cosine / log-softmax reduction.  Cosine
similarity is scale-invariant, so mask-count denominators and the global
2^-6 scale on gam cancel.

v3 layout: sequences are packed to SLC=288 tokens (the data's max active
count is 284, so 384-padding wasted 25% of the h payload, which is shipped
twice).  Seq boundaries sit at fixed offsets 288*s, so chunk/seq incidence
structure is compile-time constant and identical on all 8 cores (one SPMD
program).  Boundary chunks are handled data-side: per-incidence abx/ab2
with foreign-token rows zeroed, so every matmul runs full 128 partitions.

The D-pass uses one wide [40, 768] psum accumulation over all 23 chunks
(lhsT = 40 per-seq mask columns: 20 static a/b pairs + 20 gam-scaled pairs
written on device), replacing per-seq 32-row strips: 23x768 col-streams
instead of 30x768, and a 61KB output.

DMA: one big per-core DRAM row [128, MROW], loaded via ~17 ordered
dma_starts all issued from the otherwise-idle sync sequencer, interleaving
hx/hxt chunk-groups so the G-pass starts ~1.6us in and the U-pass trails
by two chunks.
"""

import os

import numpy as np

B_SEQ = 80
L = 512
H = 768
SAMPLES = 10
T = 16
VIEW_RANGE = 2
TEMP = 0.2
AVG_EPS = 1e-6
COS_EPS = 1e-8

N_CORES = 8
SPC = SAMPLES  # sequences per core = one dialogue
P = 128
HB = H // P  # 6 h-blocks
TT = 2 * T  # 32
GSC = 2.0**-6  # keeps a*gam within fp8e4 range; cancels in the cosine
SLC_DEFAULT = 288  # per-seq packed token budget (max active count is 284)

_CACHE: dict = {}


def _structure(slc):
    """Fixed chunk/sequence incidence structure for a given per-seq budget."""
    nt = slc * SPC
    nch = (nt + P - 1) // P
    ntp = nch * P
    chunks_of_seq = [
        list(range((slc * s) // P, (slc * (s + 1) - 1) // P + 1)) for s in range(SPC)
    ]
    incs = []  # chunk-major (c, s) pairs
    inc_of = {}
    for c in range(nch):
        for s in range(SPC):
            if c in chunks_of_seq[s]:
                inc_of[(c, s)] = len(incs)
                incs.append((c, s))
    s_last_of_chunk = {c: max(s for (cc, s) in incs if cc == c) for c in range(nch)}
    ni = len(incs)
    off_abx = 0
    off_ab2 = ni * TT
    off_abd = 2 * ni * TT
    off_hx = off_abd + nch * 40
    off_hxt = off_hx + nch * H
    mrow = off_hxt + nch * H
    return dict(
        slc=slc, nt=nt, nch=nch, ntp=ntp, chunks_of_seq=chunks_of_seq,
        incs=incs, inc_of=inc_of, s_last_of_chunk=s_last_of_chunk, ni=ni,
        off_abx=off_abx, off_ab2=off_ab2, off_abd=off_abd, off_hx=off_hx,
        off_hxt=off_hxt, mrow=mrow,
    )


def _build_nc(st):
    from contextlib import ExitStack

    import concourse.bacc as bacc
    import concourse.mybir as mybir
    import concourse.tile as tile

    f32 = mybir.dt.float32
    bf16 = mybir.dt.bfloat16
    f8 = mybir.dt.float8e4

    nch, ni = st["nch"], st["ni"]
    incs, inc_of = st["incs"], st["inc_of"]
    chunks_of_seq = st["chunks_of_seq"]
    s_last_of_chunk = st["s_last_of_chunk"]
    OFF_ABX, OFF_AB2, OFF_ABD = st["off_abx"], st["off_ab2"], st["off_abd"]
    OFF_HX, OFF_HXT, MROW = st["off_hx"], st["off_hxt"], st["mrow"]

    nc = bacc.Bacc(
        "TRN2",
        debug=False,
        enable_asserts=False,
        target_bir_lowering=False,
    )

    mg = nc.dram_tensor("mg", [P, MROW], f8, kind="ExternalInput").ap()
    out = nc.dram_tensor("out", [4 * SPC, H], bf16, kind="ExternalOutput").ap()

    with tile.TileContext(nc) as tc, ExitStack() as ctx:
        mgp = ctx.enter_context(tc.tile_pool(name="mgp", bufs=1))
        gtp = ctx.enter_context(tc.tile_pool(name="gtp", bufs=5))
        scp = ctx.enter_context(tc.tile_pool(name="scp", bufs=4))
        gmp = ctx.enter_context(tc.tile_pool(name="gmp", bufs=1))
        osp = ctx.enter_context(tc.tile_pool(name="osp", bufs=1))
        pgp = ctx.enter_context(tc.tile_pool(name="pgp", bufs=3, space="PSUM"))
        pup = ctx.enter_context(tc.tile_pool(name="pup", bufs=4, space="PSUM"))
        pdp = ctx.enter_context(tc.tile_pool(name="pdp", bufs=1, space="PSUM"))

        mgt = mgp.tile([P, MROW], f8, name="mg", tag="mg")
        gam = gmp.tile([P, nch], f32, name="gam", tag="ga")

        # ---- DMA schedule ----
        # Each issuing sequencer owns a subset of the 16 HWDGE rings, so
        # starts are spread round-robin across four engines to engage all
        # rings; each engine's ~670ns DGE configs also run in parallel.
        # First four starts cover everything G(0)/U(0) need.
        engs = [nc.sync, nc.scalar]
        ei = 0

        def span(a, b):
            nonlocal ei
            engs[ei % 2].dma_start(mgt[:, a:b], mg[:, a:b])
            ei += 1

        span(OFF_ABX, OFF_ABD)  # abx + ab2 masks
        span(OFF_HX, OFF_HX + 3 * H)  # hx chunks 0-2 (all of seq 0)
        span(OFF_HXT, OFF_HXT + 2 * H)  # hxt chunks 0-1
        nc.gpsimd.dma_start(mgt[:, OFF_ABD:OFF_HX], mg[:, OFF_ABD:OFF_HX])
        hx_groups = [(3, 6), (6, 10), (10, 14), (14, 18), (18, nch)]
        hxt_groups = [(2, 5), (5, 9), (9, 13), (13, 17), (17, nch)]
        for (ha, hb_), (ta, tb) in zip(hx_groups, hxt_groups):
            span(OFF_HX + ha * H, OFF_HX + hb_ * H)
            span(OFF_HXT + ta * H, OFF_HXT + tb * H)

        # ---- compute passes ----
        def g_pass(s):
            pg = pgp.tile([P, HB * TT], f32, name=f"pg{s}", tag="pg")
            cs = chunks_of_seq[s]
            for hb in range(HB):
                for k, c in enumerate(cs):
                    nc.tensor.matmul(
                        pg[:, hb * TT : (hb + 1) * TT],
                        mgt[:, OFF_HX + c * H + hb * P : OFF_HX + c * H + (hb + 1) * P],
                        mgt[:, OFF_ABX + inc_of[(c, s)] * TT : OFF_ABX + (inc_of[(c, s)] + 1) * TT],
                        start=(k == 0),
                        stop=(k == len(cs) - 1),
                        skip_group_check=True,
                    )
            gt = gtp.tile([P, HB * TT], f8, name=f"gt{s}", tag="gt")
            half = HB * TT // 2
            nc.vector.tensor_copy(gt[:, 0:half], pg[:, 0:half])
            nc.scalar.copy(gt[:, half:], pg[:, half:])
            return gt

        def u_pass(c, gts):
            """U + gam + abd scale for chunk c (all incidences fused).

            A boundary chunk's two incidences have disjoint token rows, so
            their gam columns are combined by one STT over the paired pu
            tile (their ab2 columns are adjacent), and one tensor_scalar
            mul covers both seqs' adjacent abd columns.  Foreign seqs' dyn
            cols stay at the zeros DMA shipped.
            """
            ss = s_list(c)
            i0 = inc_of[(c, ss[0])]
            w = len(ss) * TT
            pu = pup.tile([P, 2 * TT], f32, name=f"pu{c}", tag="pu")
            for j, s in enumerate(ss):
                for hb in range(HB):
                    nc.tensor.matmul(
                        pu[:, j * TT : (j + 1) * TT],
                        mgt[:, OFF_HXT + c * H + hb * P : OFF_HXT + c * H + (hb + 1) * P],
                        gts[s][:, hb * TT : (hb + 1) * TT],
                        start=(hb == 0),
                        stop=(hb == HB - 1),
                        skip_group_check=True,
                    )
            scr = scp.tile([P, 2 * TT], bf16, name=f"sc{c}", tag="sc")
            nc.vector.scalar_tensor_tensor(
                out=scr[:, 0:w],
                in0=pu[:, 0:w],
                scalar=1.0,
                in1=mgt[:, OFF_AB2 + i0 * TT : OFF_AB2 + i0 * TT + w],
                op0=mybir.AluOpType.bypass,
                op1=mybir.AluOpType.mult,
                accum_out=gam[:, c : c + 1],
            )
            base = OFF_ABD + c * 40
            s0 = ss[0]
            nc.gpsimd.tensor_scalar_mul(
                mgt[:, base + 20 + 2 * s0 : base + 20 + 2 * s0 + 2 * len(ss)],
                mgt[:, base + 2 * s0 : base + 2 * s0 + 2 * len(ss)],
                gam[:, c : c + 1],
            )

        def s_list(c):
            return sorted(s for (cc, s) in incs if cc == c)

        pd0 = pdp.tile([4 * SPC, 512], f32, name="pd0", tag="pd")
        pd1 = pdp.tile([4 * SPC, H - 512], f32, name="pd1", tag="pd")

        def d_pass(c):
            for pd, n0, n1 in ((pd0, 0, 512), (pd1, 512, H)):
                nc.tensor.matmul(
                    pd,
                    mgt[:, OFF_ABD + c * 40 : OFF_ABD + (c + 1) * 40],
                    mgt[:, OFF_HX + c * H + n0 : OFF_HX + c * H + n1],
                    start=(c == 0),
                    stop=(c == nch - 1),
                    skip_group_check=True,
                )

        # ---- software pipeline ----
        # G is seq-major; U(c) fires once the last seq overlapping c has its
        # gt ready; D trails U by 2 chunks to hide the DVE/gpsimd gam chain.
        gts = {}
        u_done = 0
        d_done = 0

        def drain_u(limit):
            nonlocal u_done
            while u_done < limit:
                u_pass(u_done, gts)
                u_done += 1

        def drain_d(limit):
            nonlocal d_done
            while d_done < limit:
                d_pass(d_done)
                d_done += 1

        for s in range(SPC):
            gts[s] = g_pass(s)
            ready = max(c for c in range(nch) if s_last_of_chunk[c] <= s) + 1
            drain_u(ready)
            drain_d(max(0, u_done - 2))
        drain_u(nch)
        drain_d(nch)

        ot = osp.tile([4 * SPC, H], bf16, name="ot", tag="ot")
        nc.scalar.copy(ot[:, 0:512], pd0)
        nc.vector.tensor_copy(ot[:, 512:H], pd1)
        nc.sync.dma_start(out, ot)

    nc.compile()
    return nc


def _prep_core_inputs(hidden_states, attention_mask, role_ids, turn_ids, st):
    """Per-core packed input rows: index-only mask prep + fp8 quantize."""
    import ml_dtypes

    f8 = ml_dtypes.float8_e4m3

    slc, nch, ntp, ni = st["slc"], st["nch"], st["ntp"], st["ni"]
    incs = st["incs"]

    active = attention_mask != 0
    counts = active.sum(-1)
    assert counts.max() <= slc, f"active tokens {counts.max()} exceed SLC={slc}"
    # stable-sort active tokens to the front; padded positions carry zero masks
    sel = np.argsort(~active, axis=1, kind="stable")[:, :slc]

    am = np.take_along_axis(active, sel, axis=1).astype(np.float32)
    ro = np.take_along_axis(role_ids, sel, axis=1)
    tu = np.take_along_axis(turn_ids, sel, axis=1)
    hc = np.take_along_axis(hidden_states, sel[..., None], axis=1)

    a = am * (ro == 0)
    b = am * (ro == 1)
    onehot = (tu[..., None] == np.arange(T, dtype=tu.dtype)).astype(np.float32)
    A1 = onehot * a[..., None]
    B1 = onehot * b[..., None]
    band = (
        np.abs(np.arange(T)[:, None] - np.arange(T)[None, :]) <= VIEW_RANGE
    ).astype(np.float32)
    ABX = np.concatenate([B1 @ band, A1 @ band], axis=-1)  # [B, slc, 32]
    AB2 = np.concatenate([A1, B1], axis=-1)

    hq = hc.astype(f8)  # quantize once; all views share the same values
    hq32 = hq.astype(np.float32)

    in_maps = []
    for core in range(N_CORES):
        sl = slice(core * SPC, (core + 1) * SPC)
        # packed token stream [ntp, ...] with seq s at rows slc*s
        hs = np.zeros((ntp, H), np.float32)
        abxs = np.zeros((ntp, TT), np.float32)
        ab2s = np.zeros((ntp, TT), np.float32)
        as_ = np.zeros(ntp, np.float32)
        bs_ = np.zeros(ntp, np.float32)
        for s in range(SPC):
            g = core * SPC + s
            hs[slc * s : slc * (s + 1)] = hq32[g]
            abxs[slc * s : slc * (s + 1)] = ABX[g]
            ab2s[slc * s : slc * (s + 1)] = AB2[g]
            as_[slc * s : slc * (s + 1)] = a[g]
            bs_[slc * s : slc * (s + 1)] = b[g]

        # per-incidence abx/ab2: chunk c rows, foreign-seq rows zeroed
        abx_i = np.zeros((P, ni * TT), np.float32)
        ab2_i = np.zeros((P, ni * TT), np.float32)
        for i, (c, s) in enumerate(incs):
            rows = np.arange(c * P, (c + 1) * P)
            inseq = (rows >= slc * s) & (rows < slc * (s + 1))
            abx_i[:, i * TT : (i + 1) * TT] = abxs[rows] * inseq[:, None]
            ab2_i[:, i * TT : (i + 1) * TT] = ab2s[rows] * inseq[:, None]

        # abd: [128, nch*40]; per chunk: 20 static cols (a,b)*GSC at 2s,
        # 20 dynamic cols (device-written)
        abd = np.zeros((P, nch * 40), np.float32)
        for c in range(nch):
            rows = np.arange(c * P, (c + 1) * P)
            svec = rows // slc  # owning seq per row (pad rows clamp to 9)
            svec = np.minimum(svec, SPC - 1)
            cols0 = c * 40 + 2 * svec
            abd[np.arange(P), cols0] = as_[rows] * GSC
            abd[np.arange(P), cols0 + 1] = bs_[rows] * GSC

        # hx: [128, nch*768]; hx[p, c*H + j] = h[c*128+p, j]
        hx = hs.reshape(nch, P, H).transpose(1, 0, 2).reshape(P, nch * H)
        # hxt: [128, nch*768]; hxt[p, c*H + hb*P + t] = h[c*128+t, hb*P+p]
        hxt = (
            hs.reshape(nch, P, HB, P)
            .transpose(3, 0, 2, 1)  # [p, c, hb, t]
            .reshape(P, nch * H)
        )

        mgall = np.concatenate(
            [
                abx_i.astype(f8).view(np.uint8),
                ab2_i.astype(f8).view(np.uint8),
                abd.astype(f8).view(np.uint8),
                hx.astype(f8).view(np.uint8),
                hxt.astype(f8).view(np.uint8),
            ],
            axis=-1,
        ).view(f8)
        assert mgall.shape == (P, st["mrow"])
        in_maps.append({"mg": np.ascontiguousarray(mgall)})

    # cheap integrity reference: qs/rs rows recomputed on host from the same fp8 h
    qs_ref = np.einsum("bl,blh->bh", a, hq32) * GSC
    rs_ref = np.einsum("bl,blh->bh", b, hq32) * GSC
    return in_maps, a.sum(-1), b.sum(-1), qs_ref, rs_ref


def _outputs_ok(vecs, qs_ref, rs_ref):
    """Detect corrupted device runs: finite outputs + qs/rs rows match host."""
    if not np.isfinite(vecs).all():
        return False
    for got, ref in ((vecs[:, 0], qs_ref), (vecs[:, 1], rs_ref)):
        num = np.linalg.norm(got - ref, axis=-1)
        den = np.linalg.norm(ref, axis=-1) + 1e-6
        if (num / den).max() > 0.05:
            return False
    return True


def _finalize(vecs, labels, na, nb):
    """Host-side O(B*H) reduction: cosine, log-softmax, label-weighted loss.

    vecs: [B, 4, H] rows [qs, rs, qc, rc] (uniform scales cancel in cosine).
    """
    vecs = vecs.astype(np.float64)
    qs = vecs[:, 0] / (na + AVG_EPS)[:, None]
    rs = vecs[:, 1] / (nb + AVG_EPS)[:, None]
    qc = vecs[:, 2] / (nb + AVG_EPS)[:, None]
    rc = vecs[:, 3] / (na + AVG_EPS)[:, None]

    def cos(x, y):
        nx = np.maximum(np.linalg.norm(x, axis=-1), COS_EPS)
        ny = np.maximum(np.linalg.norm(y, axis=-1), COS_EPS)
        return (x * y).sum(-1) / (nx * ny)

    logit_q = (cos(qs, qc) / TEMP).reshape(-1, SAMPLES)
    logit_r = (cos(rs, rc) / TEMP).reshape(-1, SAMPLES)

    def lsm(x):
        m = x.max(-1, keepdims=True)
        e = np.exp(x - m)
        return x - m - np.log(e.sum(-1, keepdims=True))

    lab = labels.astype(np.float64)
    loss_q = -np.mean(lsm(logit_q) * lab)
    loss_r = -np.mean(lsm(logit_r) * lab)
    return np.float32(loss_r + loss_q)


def kernel(hidden_states, labels, attention_mask, role_ids, turn_ids):
    import time

    from concourse.bass_utils import run_bass_kernel_spmd

    attention_mask = np.asarray(attention_mask)
    maxc = int((attention_mask != 0).sum(-1).max())
    slc = max(SLC_DEFAULT, -(-maxc // 32) * 32)
    if ("nc", slc) not in _CACHE:
        _CACHE[("nc", slc)] = _build_nc(_structure(slc))
    nc = _CACHE[("nc", slc)]
    st = _structure(slc)

    in_maps, na, nb, qs_ref, rs_ref = _prep_core_inputs(
        np.asarray(hidden_states),
        attention_mask,
        np.asarray(role_ids),
        np.asarray(turn_ids),
        st,
    )
    trace = bool(os.environ.get("BASS_KERNEL_TRACE"))

    # the axon/NRT path very occasionally drops a run; validate cheaply and retry
    vecs = None
    for attempt in range(3):
        try:
            res = run_bass_kernel_spmd(
                nc, in_maps, core_ids=list(range(N_CORES)), trace=trace
            )
            cand = np.concatenate(
                [res.results[c]["out"].reshape(SPC * 2, 2, H) for c in range(N_CORES)],
                axis=0,
            )  # per core rows: [2s]=qs, [2s+1]=rs then [20+2s]=qc, [21+2s]=rc
            cand = (
                cand.reshape(N_CORES, 2, SPC, 2, H)
                .transpose(0, 2, 3, 1, 4)
                .reshape(B_SEQ, 4, H)
            )  # -> [b, (qs, qc, rs, rc)? see below]
            # rows come out as [qs, qc, rs, rc]; reorder to [qs, rs, qc, rc]
            cand = cand[:, [0, 2, 1, 3], :]
        except Exception as e:
            import traceback

            print(f"[kernel] attempt {attempt} failed: {type(e).__name__}: {e}")
            traceback.print_exc()
            if attempt == 2:
                raise
            time.sleep(2.0)
            continue
        vecs = cand
        if _outputs_ok(cand, qs_ref, rs_ref):
            break
    if trace:
        _CACHE["last_results"] = res
        print(
            f"[kernel] exec_time_ns={res.exec_time_ns} "
            f"mean_exec_time_ns={res.mean_exec_time_ns}"
        )
    return _finalize(vecs, np.asarray(labels), na, nb)
